# revision 67
# baseline (speedup 1.0000x reference)
"""Trainium2 Bass kernel for masked edge-softmax attention aggregation.

  score[j] = (inputs @ H_v)[j]
  E[i,j]   = exp(adj[i,j]*score[j]) if adj[i,j]!=0 else 0
  out      = (E @ inputs) / rowsum(E)

Sharding/staging strategy (host side, layout only — no FLOPs of the
operator are done on the host):
  - adj rows are sharded over 8 cores (1250 rows each); each shard is
    staged PRE-TRANSPOSED as adjT [N, R] and converted to fp16, halving
    the dominant HBM traffic (50MB -> 25MB per core) and eliminating all
    on-device PE transposes.
  - inputs are staged replicated as a ready-to-DMA SBUF image
    aug_img [128, NJ*W] fp16 = per j-block [x_block | ones-column], used
    both as the matmul RHS and (with H_v) to compute score on device.
  - H_v is staged replicated across partitions [128, D] fp16.

Per-core program (no collectives):
  prologue: ACT Exp-table warm-up; aug_img DMAd in 16 small chunks (the
            later ones streamed inside the main loop so they don't flood
            the SP DMA queue and starve slab prefetch); per chunk
            score = sum_d aug*hv (multiply on Pool, reduce on DVE,
            reduces issued just-in-time in the main loop so they don't
            clog DVE's 8-deep in-order exec window).
  main loop over 79 j-blocks (adjT slabs [128, 1250] fp16):
            ACT:  e0 = Exp(score_p * a)        (1 op, FD=1250, 1227ns)
            DVE:  m  = (a > 0)                 (is_gt, 4x mode, 386ns)
            DVE (every 5th block Pool): e = e0 * m   (exact mask)
            PE:   acc_it[ri, 129] += e_chunk.T @ [x_b | 1]  (10 matmuls,
                  PSUM-resident accumulators, 3 slots per 512-col bank;
                  start/stop only on the first/last slot of each bank —
                  they clear/release the whole 2KB zero region)
  epilogue: per bank: one strided reciprocal of the 3 rowsum columns,
            scales split ACT/DVE, one merged output DMA per bank.

Engine busy: ACT ~98us (ceiling) > DVE ~92 > DMA 78.6 > Pool > PE.
The 79-exp ACT stream runs gap-free (79 x 1227ns); remaining overhead
is ~4.7us prologue latency and ~8.1us epilogue drain, both within
~0.5us of their structural floors.
Measured 109799 ns (cost-model timeline), rel err 4.5e-4.

Explored and rejected (for future reference):
  - Batched exp via DVE prescale (+PE e0+m-1 masking w/ S-fixup): drops
    ACT to ~85us but the prescale+mask (2x386ns) load DVE/PE/Pool to
    93-96% of the batched pace; in-order queue friction measured ~14us
    already at 87% loads (best pipelined sim: 123.8us). Dead end under
    this cost model.
  - pow-on-DVE exp offload, TensorTensorReduce, ALU.divide epilogue,
    Pool tensor_scalar-with-ptr: all fail to compile/lower on hw.
  - fp16 output: descriptor runs drop to 256B -> 2x DMA latency
    multiplier cancels the byte savings exactly.
"""

import os

import numpy as np

import concourse.bacc as bacc
import concourse.bass as bass
import concourse.mybir as mybir
import concourse.tile as tile
from concourse.bass_utils import run_bass_kernel_spmd

N = 10000
D = 128
NCORES = 8
R = N // NCORES          # 1250 rows per core
P = 128
NJ = (N + P - 1) // P    # 79 j-blocks, last has 16 rows
NI = (R + P - 1) // P    # 10 i-tiles, last has 98 rows
W = D + 1                # aug width (inputs | ones)

F32 = mybir.dt.float32
F16 = mybir.dt.float16
AF = mybir.ActivationFunctionType
ALU = mybir.AluOpType

# every k-th block's mask-apply multiply goes to Pool, from POOL_MULT_START on
# (before that Pool is still busy with the prologue score multiplies)
POOL_MULT_PERIOD = int(os.environ.get("POOL_MULT_PERIOD", "5"))
POOL_MULT_START = int(os.environ.get("POOL_MULT_START", "30"))
SLAB_BUFS = int(os.environ.get("SLAB_BUFS", "8"))
WORK_BUFS = int(os.environ.get("WORK_BUFS", "6"))
# DVE reduce for score chunk c is issued this many blocks before first use
REDUCE_LEAD = int(os.environ.get("REDUCE_LEAD", "9"))

# first chunk tiny (tensor_tensor_reduce per block) so block 0 starts ASAP
SCORE_CHUNKS = [(5 * k, 5) for k in range(15)] + [(75, 4)]


def _pb(b):
    return P if b < NJ - 1 else N - (NJ - 1) * P


def _ri(i):
    return P if i < NI - 1 else R - (NI - 1) * P


def build_nc():
    nc = bacc.Bacc("TRN2", target_bir_lowering=False, debug=False, num_devices=NCORES)

    adjt = nc.dram_tensor("adjt_shard", [N, R], F16, kind="ExternalInput")
    aug_img = nc.dram_tensor("aug_img", [P, D + NJ * W], F16, kind="ExternalInput")
    out_s = nc.dram_tensor("out_shard", [R, D], F32, kind="ExternalOutput")

    with tile.TileContext(nc) as tc:
        with (
            tc.tile_pool(name="const", bufs=1) as constp,
            tc.tile_pool(name="slab", bufs=SLAB_BUFS) as slabp,
            tc.tile_pool(name="work", bufs=WORK_BUFS) as workp,
            tc.tile_pool(name="fix", bufs=10) as fixp,
            tc.tile_pool(name="psumacc", bufs=1, space="PSUM") as psumaccp,
        ):
            # ---------------- constants / prologue ----------------
            hv_aug_sb = constp.tile([P, D + NJ * W], F16)
            hv_sb = hv_aug_sb[:, 0:D]
            aug_sb = hv_aug_sb[:, D : D + NJ * W]
            aug3 = aug_sb.rearrange("p (b w) -> p b w", w=W)
            score_sb = constp.tile([P, NJ], F32)

            def load_aug_chunk(c0, nb, with_hv=False):
                if with_hv:
                    # hv rides at the head of aug_img; one DMA covers both
                    nc.sync.dma_start(
                        hv_aug_sb[:, 0 : D + nb * W], aug_img[:, 0 : D + nb * W]
                    )
                else:
                    nc.sync.dma_start(
                        aug_sb[:, c0 * W : (c0 + nb) * W],
                        aug_img[:, D + c0 * W : D + (c0 + nb) * W],
                    )

            stmps = {}

            def score_mult(ci, engine, two_stage=False):
                # stmp[p, b, d] = aug[p, b, d] * hv[d]; optionally also fold
                # the d-halves together on the same engine so the DVE reduce
                # only has to sum 64 columns per block
                c0, nb = SCORE_CHUNKS[ci]
                stmp = constp.tile([P, 12 * D], F16, tag=f"stmp{ci}", name=f"stmp{ci}")
                hv_rep = (
                    hv_sb
                    .rearrange("p (o d) -> p o d", o=1)
                    .broadcast_to([P, nb, D])
                )
                s3 = stmp[:, 0 : nb * D].rearrange("p (b d) -> p b d", d=D)
                engine.tensor_tensor(s3, aug3[:, c0 : c0 + nb, 0:D], hv_rep, ALU.mult)
                if two_stage:
                    h = D // 2
                    engine.tensor_tensor(
                        s3[:, :, 0:h], s3[:, :, 0:h], s3[:, :, h:D], ALU.add
                    )
                stmps[ci] = (stmp, two_stage)

            def score_reduce(ci):
                # score[p, b] = sum_d stmp[p, b, d]
                c0, nb = SCORE_CHUNKS[ci]
                stmp, two_stage = stmps.pop(ci)
                dd = D // 2 if two_stage else D
                nc.vector.tensor_reduce(
                    score_sb[:, c0 : c0 + nb],
                    stmp[:, 0 : nb * D]
                    .rearrange("p (b d) -> p b d", d=D)[:, :, 0:dd],
                    axis=mybir.AxisListType.X,
                    op=ALU.add,
                )

            def load_slab(b):
                pb = _pb(b)
                sl = slabp.tile([P, R], F16, tag="slab", name=f"sl{b}")
                nc.sync.dma_start(sl[0:pb, :], adjt[b * P : b * P + pb, :])
                return sl

            # DMA order: tiny aug chunk 0, hv, first slabs, remaining aug
            # chunks interleaved with more slab prefetches. Pool does all the
            # score multiplies up front (it is otherwise idle early); the DVE
            # reduces are issued just-in-time inside the main loop so they
            # don't clog DVE's in-order exec window.
            # warm up the ACT Exp table immediately (1.3us load) so it's off
            # the first real exp's critical path
            warm = constp.tile([1, 1], F32)
            nc.vector.memset(warm[:, :], 0.0)
            warm2 = constp.tile([1, 1], F32)
            nc.scalar.activation(warm2[:, :], warm[:, :], AF.Exp)

            slabs = {}
            load_aug_chunk(*SCORE_CHUNKS[0], with_hv=True)
            slabs[0] = load_slab(0)
            # chunk 0 small on DVE: lowest latency to the first exp
            score_mult(0, nc.vector, two_stage=True)
            score_reduce(0)
            slabs[1] = load_slab(1)
            for ci in (1, 2):
                load_aug_chunk(*SCORE_CHUNKS[ci])
                slabs[ci + 1] = load_slab(ci + 1)
                score_mult(ci, nc.gpsimd)

            # later aug chunks stream in during the loop so the 16 up-front
            # DMAs don't flood the SP queue and starve slab prefetch
            chunk_at_block = {
                max(0, SCORE_CHUNKS[ci][0] - 12): ci
                for ci in range(3, len(SCORE_CHUNKS))
            }
            reduce_at_block = {
                max(0, SCORE_CHUNKS[ci][0] - REDUCE_LEAD): ci
                for ci in range(1, len(SCORE_CHUNKS))
            }

            # PSUM accumulators: 10 i-tiles, 3 slots of 129 f32 per bank tile
            accs = [
                psumaccp.tile([P, 512], F32, tag=f"accb{t}", name=f"accb{t}")
                for t in range(4)
            ]

            # slot stride 136 keeps each accumulator 32B-aligned in the PSUM
            # bank; odd strides (129) corrupt the neighbouring slot's columns
            def acc_ap(it, ri):
                t, s = divmod(it, 3)
                return accs[t][0:ri, s * 136 : s * 136 + W]

            # ---------------- main loop ----------------
            for b in range(NJ):
                pb = _pb(b)
                if b in chunk_at_block:
                    ci = chunk_at_block[b]
                    load_aug_chunk(*SCORE_CHUNKS[ci])
                    score_mult(ci, nc.gpsimd)
                if b in reduce_at_block:
                    score_reduce(reduce_at_block[b])
                sl = slabs.pop(b) if b in slabs else load_slab(b)
                e0 = workp.tile([P, R], F16, tag="e0")
                nc.scalar.activation(
                    e0[0:pb, :],
                    sl[0:pb, :],
                    AF.Exp,
                    bias=0.0,
                    scale=score_sb[0:pb, b : b + 1],
                )
                m = workp.tile([P, R], F16, tag="m")
                nc.vector.tensor_scalar(
                    m[0:pb, :], sl[0:pb, :], 0.0, None, ALU.is_gt
                )
                e = workp.tile([P, R], F16, tag="e")
                eng = (
                    nc.gpsimd
                    if (
                        POOL_MULT_PERIOD
                        and b >= POOL_MULT_START
                        and b % POOL_MULT_PERIOD == POOL_MULT_PERIOD - 1
                    )
                    else nc.vector
                )
                if b == NJ - 1:
                    # split the last block's mask-apply so the first matmuls
                    # start ~400ns earlier in the drain tail
                    h = 5 * P
                    eng.tensor_tensor(
                        e[0:pb, 0:h], e0[0:pb, 0:h], m[0:pb, 0:h], ALU.mult
                    )
                    eng.tensor_tensor(
                        e[0:pb, h:R], e0[0:pb, h:R], m[0:pb, h:R], ALU.mult
                    )
                else:
                    eng.tensor_tensor(e[0:pb, :], e0[0:pb, :], m[0:pb, :], ALU.mult)
                # start/stop are bank-granular (they clear / release the whole
                # 2KB zero region), so only the first slot of each bank may
                # start and only the last slot may stop
                for it in range(NI):
                    ri = _ri(it)
                    t, s = divmod(it, 3)
                    first_in_bank = s == 0
                    last_in_bank = (s == 2) or (it == NI - 1)
                    nc.tensor.matmul(
                        acc_ap(it, ri),
                        e[0:pb, it * P : it * P + ri],
                        aug3[0:pb, b, :],
                        start=(b == 0) and first_in_bank,
                        stop=(b == NJ - 1) and last_in_bank,
                    )

            # ---------------- epilogue ----------------
            # one wide output staging tile; per PSUM bank: 3 fixups then a
            # single merged DMA (fewer serialized HWDGE/SEQ slots in the tail)
            osb = fixp.tile([P, NI * D], F32, tag="osb", bufs=1)
            osb3 = osb[:, :].rearrange("p (i d) -> p i d", d=D)
            for t in (0, 1, 2, 3):
                its = [it for it in range(NI) if it // 3 == t]
                # one strided reciprocal covers all of this bank's rowsum
                # columns (at 128, 264, 400); then one scale per i-tile
                ns = len(its)
                rec = fixp.tile([P, 4], F32, tag="rec")
                nc.vector.reciprocal(
                    rec[0:P, 0:ns],
                    accs[t][0:P, D : D + 1 + 136 * (ns - 1) : 136][0:P, 0:ns]
                    if ns > 1
                    else accs[t][0:P, D : D + 1],
                )
                for k, it in enumerate(its):
                    ri = _ri(it)
                    a = acc_ap(it, ri)
                    if it % 2 == 0:
                        # ACT is idle after the final exp; take half the scales
                        nc.scalar.activation(
                            osb3[0:ri, it, :],
                            a[0:ri, 0:D],
                            AF.Copy,
                            bias=0.0,
                            scale=rec[0:ri, k : k + 1],
                        )
                    else:
                        nc.vector.tensor_scalar(
                            osb3[0:ri, it, :],
                            a[0:ri, 0:D],
                            rec[0:ri, k : k + 1],
                            None,
                            ALU.mult,
                        )
                it0 = its[0]
                dma_q = nc.sync
                rows = sum(_ri(it) for it in its)
                if rows == len(its) * P:
                    dma_q.dma_start(
                        out_s[it0 * P : it0 * P + rows, :].rearrange(
                            "(i p) d -> p i d", p=P
                        ),
                        osb3[:, it0 : it0 + len(its), :],
                    )
                else:
                    nfull = rows // P
                    if nfull:
                        dma_q.dma_start(
                            out_s[it0 * P : (it0 + nfull) * P, :].rearrange(
                                "(i p) d -> p i d", p=P
                            ),
                            osb3[:, it0 : it0 + nfull, :],
                        )
                    rpart = rows - nfull * P
                    dma_q.dma_start(
                        out_s[(it0 + nfull) * P : (it0 + nfull) * P + rpart, :],
                        osb3[0:rpart, it0 + nfull, :],
                    )

    nc.compile()
    return nc


_NC = None


def _get_nc():
    global _NC
    if _NC is None:
        _NC = build_nc()
    return _NC


def _stage_inputs(inputs, adj, H_v):
    """Host-side layout staging: shard + transpose + fp16 + aug image."""
    inputs = np.asarray(inputs, dtype=np.float32)
    adj = np.asarray(adj, dtype=np.float32)
    H_v = np.asarray(H_v, dtype=np.float32)

    adj16t = np.ascontiguousarray(adj.astype(np.float16).T)  # [N, N]

    aug = np.zeros((P, D + NJ * W), dtype=np.float16)
    aug[:, 0:D] = H_v.reshape(1, D).astype(np.float16)  # hv replicated head
    inp16 = inputs.astype(np.float16)
    for b in range(NJ):
        pb = _pb(b)
        aug[0:pb, D + b * W : D + b * W + D] = inp16[b * P : b * P + pb, :]
        aug[0:pb, D + b * W + D] = np.float16(1.0)
    in_maps = [
        {
            "adjt_shard": np.ascontiguousarray(adj16t[:, c * R : (c + 1) * R]),
            "aug_img": aug,
        }
        for c in range(NCORES)
    ]
    return in_maps


def kernel(inputs, adj, H_v, _trace=False, _trace_kwargs=None):
    nc = _get_nc()
    in_maps = _stage_inputs(inputs, adj, H_v)
    kw = {}
    if _trace:
        kw = dict(trace=True, **(_trace_kwargs or {}))
    res = run_bass_kernel_spmd(nc, in_maps, list(range(NCORES)), **kw)
    if _trace:
        kernel._last_results = res
    outs = res.results
    return np.concatenate(
        [np.asarray(outs[c]["out_shard"], dtype=np.float32) for c in range(NCORES)],
        axis=0,
    )
